# revision 20
# baseline (speedup 1.0000x reference)
"""Masked-softmax attention (B=4, H=16, S=2048, D=64) for 8 Trainium2 NeuronCores.

Returns (out, p_attn) like the reference:
    scores = (Q @ K^T) / sqrt(D);  scores[mask==1] = -1e8
    p_attn = softmax(scores, axis=-1);  out = p_attn @ V

Sharding: B x H = 64 (b,h) pairs -> 8 pairs per core (core c: b = c//2,
heads h in [8*(c%2), 8*(c%2)+8)). Fully local per core, no collectives.

Per-core algorithm, all in transposed layout (softmax axis k on partitions,
so P^T feeds the second matmul directly, no on-chip transposes):
  S^T [k=128, q=1024] accumulated in PSUM (2 banks) from full-rate bf16
  matmuls per 512-half:
    [Khi;Klo]^T @ [Qhi;Qhi]  +  [Khi;Klo]^T @ [Qlo;Qlo]     (split-bf16 QK^T:
        K^T = Khi+Klo and Q = Qhi+Qlo exact to ~2^-16; fp32 PSUM accumulate)
    (-1e8 * I128)^T @ maskT_tile                             (adds -1e8*mask)
  P_un = exp(S^T)  (ACT, PSUM->SBUF, fp16 out)               masked -> 0
  acc[65, 1024] += Vp_tile^T @ P_un  over 16 k-tiles (fp16 chains; Vp = [V|1]
                                                      so acc row 64 = denom)
  recip = exp(-ln(denom))  (ACT, same table set as exp -> no table reloads)
  partition-broadcast recip (GpSimd);  outT = acc[0:64] * recip (f32)
  pT tiles = P_un * recip -> fp16 -> DMA (host converts to f32)
Host packs/splits operands and reassembles/transposes outputs.
"""

import numpy as np
import ml_dtypes

import concourse.bass as bass
import concourse.tile as tile
from concourse import bacc, mybir
from concourse.bass_utils import run_bass_kernel_spmd

B, H, S, D = 4, 16, 2048, 64
NCORES = 8
HPC = 8          # heads per core
QC = 1024        # q-chunk for ACT/DVE ops (2 PSUM banks)
NQC = S // QC    # 2
QH = 512         # matmul free-dim half (1 PSUM bank)
KT = 128         # k-tile (partition dim)
NKT = S // KT    # 16
NEG = -1.0e8

_cache = {}


def build_program(reps: int = 1):
    f32, bf16, fp16 = mybir.dt.float32, mybir.dt.bfloat16, mybir.dt.float16
    nc = bacc.Bacc("TRN2", target_bir_lowering=False, debug=False)

    kl_d = nc.dram_tensor("kl", [HPC, 2 * D, S], bf16, kind="ExternalInput").ap()
    qhi_d = nc.dram_tensor("qhi", [HPC, D, S], bf16, kind="ExternalInput").ap()
    qlo_d = nc.dram_tensor("qlo", [HPC, D, S], bf16, kind="ExternalInput").ap()
    vp_d = nc.dram_tensor("vp", [HPC, S, D + 1], fp16, kind="ExternalInput").ap()
    mt_d = nc.dram_tensor("mt", [S, S], bf16, kind="ExternalInput").ap()
    id_d = nc.dram_tensor("ident", [128, 128], bf16, kind="ExternalInput").ap()
    pT_d = nc.dram_tensor("pT", [HPC, S, S], fp16, kind="ExternalOutput").ap()
    ot_d = nc.dram_tensor("outT", [HPC, D, S], f32, kind="ExternalOutput").ap()
    den_d = nc.dram_tensor("den", [HPC, S], f32, kind="ExternalOutput").ap()

    Exp = mybir.ActivationFunctionType.Exp
    Ln = mybir.ActivationFunctionType.Ln
    # Make Exp resolve to the natural_log_exp_and_others table set (which has
    # BOTH ln and exp) so the per-chunk Ln for the reciprocal never forces an
    # ACT table reload. get_activation_tables is cached and returns the same
    # dict object, so removing Exp from exp_and_others changes the pick while
    # keeping every act_func_set_id stable.
    from concourse import hw_specs
    tabs = hw_specs.get_activation_tables(nc.m.arch)
    if "natural_log_exp_and_others" in tabs and Exp in tabs["natural_log_exp_and_others"]:
        tabs.get("exp_and_others", set()).discard(Exp)

    with tile.TileContext(nc) as tc:
        with (
            tc.tile_pool(name="const", bufs=1) as cpool,
            tc.tile_pool(name="qk", bufs=2) as qkpool,
            tc.tile_pool(name="vpp", bufs=2) as vppool,
            tc.tile_pool(name="pun", bufs=24) as punpool,
            tc.tile_pool(name="pn", bufs=8) as pnpool,
            tc.tile_pool(name="small", bufs=2) as smallpool,
            tc.tile_pool(name="ps", bufs=2, space="PSUM") as pspool,
            tc.tile_pool(name="psacc", bufs=2, space="PSUM") as accpool,
        ):
            id_s = cpool.tile([128, 128], bf16)
            nc.sync.dma_start(out=id_s, in_=id_d[:, :])
            # whole transposed mask resident: [128, 16, 2048] bf16 (8.4MB).
            # Loaded per k-tile AFTER pair 0's operands so compute starts early.
            mt_s = cpool.tile([128, NKT, S], bf16)
            mt_view = mt_d.rearrange("(t p) q -> p t q", p=128)

            for _ in range(reps):
                for pair in range(HPC):
                    kl_s = qkpool.tile([128, S], bf16, tag="kl")
                    nc.sync.dma_start(out=kl_s, in_=kl_d[pair])
                    qhi_s = qkpool.tile([128, S], bf16, tag="qhi")
                    nc.sync.dma_start(out=qhi_s[0:D, :], in_=qhi_d[pair])
                    nc.sync.dma_start(out=qhi_s[D : 2 * D, :], in_=qhi_d[pair])
                    qlo_s = qkpool.tile([128, S], bf16, tag="qlo")
                    nc.sync.dma_start(out=qlo_s[0:D, :], in_=qlo_d[pair])
                    nc.sync.dma_start(out=qlo_s[D : 2 * D, :], in_=qlo_d[pair])
                    vp_s = vppool.tile([128, NKT, D + 1], fp16, tag="vp")
                    nc.sync.dma_start(
                        out=vp_s,
                        in_=vp_d[pair].rearrange("(t p) c -> p t c", p=128),
                    )
                    if pair == 0:
                        for kt_i in range(NKT):
                            nc.sync.dma_start(
                                out=mt_s[:, kt_i, :], in_=mt_view[:, kt_i, :]
                            )

                    def process_chunk(pair, q0, qw):
                        acc = accpool.tile([D + 1, QC], f32, tag="acc")
                        halves = [bass.ds(q0 + h, QH) for h in range(0, qw, QH)]
                        nh = len(halves)
                        pun_tiles = []
                        for kt_i in range(NKT):
                            ksl = bass.ts(kt_i, KT)
                            s_ps = pspool.tile([128, QC], f32, tag="s")
                            for h in range(nh):
                                nc.tensor.matmul(
                                    out=s_ps[:, h * QH:(h + 1) * QH],
                                    lhsT=kl_s[:, ksl], rhs=qhi_s[:, halves[h]],
                                    start=True, stop=False,
                                )
                            for h in range(nh):
                                nc.tensor.matmul(
                                    out=s_ps[:, h * QH:(h + 1) * QH],
                                    lhsT=kl_s[:, ksl], rhs=qlo_s[:, halves[h]],
                                    start=False, stop=False,
                                )
                            for h in range(nh):
                                nc.tensor.matmul(
                                    out=s_ps[:, h * QH:(h + 1) * QH],
                                    lhsT=id_s[:, :],
                                    rhs=mt_s[:, kt_i, halves[h]],
                                    start=False, stop=True,
                                )
                            pu = punpool.tile([128, QC], fp16, tag="pu")
                            nc.scalar.activation(pu[:, 0:qw], s_ps[:, 0:qw], Exp)
                            pun_tiles.append(pu)
                            for h in range(nh):
                                nc.tensor.matmul(
                                    out=acc[:, h * QH:(h + 1) * QH],
                                    lhsT=vp_s[:, kt_i, :],
                                    rhs=pu[:, h * QH:(h + 1) * QH],
                                    start=(kt_i == 0), stop=(kt_i == NKT - 1),
                                )

                        # one quick DVE copy frees the acc PSUM banks; out is
                        # normalized on the host (out = out_raw / den), so the
                        # reciprocal chain below only gates the pn muls.
                        o_s = smallpool.tile([D + 1, QC], f32, tag="o")
                        nc.vector.tensor_copy(o_s[:, 0:qw], acc[:, 0:qw])
                        nc.sync.dma_start(
                            out=ot_d[pair, :, bass.ds(q0, qw)], in_=o_s[0:D, 0:qw]
                        )
                        nc.gpsimd.dma_start(
                            out=den_d[pair, bass.ds(q0, qw)], in_=o_s[D : D + 1, 0:qw]
                        )
                        lnd = smallpool.tile([1, QC], f32, tag="lnd")
                        nc.scalar.activation(lnd[:, 0:qw], o_s[D : D + 1, 0:qw], Ln)
                        rc = smallpool.tile([1, QC], f32, tag="rc")
                        nc.scalar.activation(rc[:, 0:qw], lnd[:, 0:qw], Exp, scale=-1.0)
                        rep = smallpool.tile([128, QC], f32, tag="rep")
                        nc.gpsimd.partition_broadcast(rep[:, 0:qw], rc[0:1, 0:qw])
                        for kt_i in range(NKT):
                            pn = pnpool.tile([128, QC], fp16, tag="pn")
                            nc.vector.tensor_mul(
                                pn[:, 0:qw], pun_tiles[kt_i][:, 0:qw], rep[:, 0:qw]
                            )
                            nc.sync.dma_start(
                                out=pT_d[pair, bass.ts(kt_i, KT), bass.ds(q0, qw)],
                                in_=pn[:, 0:qw],
                            )

                    chunks = [(0, QC), (QC, QC)]
                    for q0, qw in chunks:
                        process_chunk(pair, q0, qw)
    nc.compile()
    return nc


def prep_inputs(query, key, value, mask):
    query = np.asarray(query, dtype=np.float32)
    key = np.asarray(key, dtype=np.float32)
    value = np.asarray(value, dtype=np.float32)
    mask = np.asarray(mask)

    scale = np.float32(1.0 / np.sqrt(D))
    qt = query.transpose(0, 1, 3, 2) * scale                  # [B,H,D,S] f32
    qhi = qt.astype(ml_dtypes.bfloat16)
    qlo = (qt - qhi.astype(np.float32)).astype(ml_dtypes.bfloat16)
    kt = np.ascontiguousarray(key.transpose(0, 1, 3, 2))      # [B,H,D,S] f32
    khi = kt.astype(ml_dtypes.bfloat16)
    klo = (kt - khi.astype(np.float32)).astype(ml_dtypes.bfloat16)
    kl = np.concatenate([khi, klo], axis=2)                   # [B,H,2D,S] bf16
    vp = np.empty((B, H, S, D + 1), np.float16)
    vp[..., :D] = value.astype(np.float16)
    vp[..., D] = 1.0
    mt = (
        mask.reshape(B, S, S).transpose(0, 2, 1).astype(ml_dtypes.bfloat16)
    )                                                         # [B,S,S] (k,q)
    ident = (np.eye(128, dtype=np.float32) * NEG).astype(ml_dtypes.bfloat16)

    in_maps = []
    for c in range(NCORES):
        b, hg = c // 2, c % 2
        hs = hg * HPC
        in_maps.append(
            {
                "kl": np.ascontiguousarray(kl[b, hs : hs + HPC]),
                "qhi": np.ascontiguousarray(qhi[b, hs : hs + HPC]),
                "qlo": np.ascontiguousarray(qlo[b, hs : hs + HPC]),
                "vp": np.ascontiguousarray(vp[b, hs : hs + HPC]),
                "mt": np.ascontiguousarray(mt[b]),
                "ident": ident,
            }
        )
    return in_maps


def assemble_outputs(results):
    p_attn_t = np.empty((B, 2, HPC, S, S), np.float32)
    out_t = np.empty((B, 2, HPC, D, S), np.float32)
    for c in range(NCORES):
        b, hg = c // 2, c % 2
        p_attn_t[b, hg] = results[c]["pT"]
        den = results[c]["den"].reshape(HPC, 1, S)
        out_t[b, hg] = results[c]["outT"] / den
    # [B, H, k, q] -> view [B, H, q, k]
    p_attn = p_attn_t.reshape(B, H, S, S).swapaxes(2, 3)
    out = np.ascontiguousarray(out_t.reshape(B, H, D, S).swapaxes(2, 3))
    return out, p_attn


def kernel(query, key, value, mask):
    if "nc" not in _cache:
        _cache["nc"] = build_program()
    nc = _cache["nc"]
    in_maps = prep_inputs(query, key, value, mask)
    res = run_bass_kernel_spmd(nc, in_maps, core_ids=list(range(NCORES)))
    return assemble_outputs(res.results)
